# revision 5
# baseline (speedup 1.0000x reference)
"""Trainium2 Bass kernel for nn_EncodingShake (VQ codebook encoding with shake).

Math (per batch b):
  Xf = X[b].reshape(D, N).T                      # (N, D), N = H*W
  sl[n,k]  = s_k*||Xf[n]-C[k]||^2 = s_k*x2[n] - 2 s_k <Xf[n],C[k]> + s_k*c2[k]
  A        = softmax_k(sl)                       # (N, K)
  E[k,d]   = sum_n A[n,k]*Xf[n,d] - (sum_n A[n,k])*C[k,d]

Sharding: data-parallel over B — 8 cores x 2 batches each; codebook/scale
replicated. No collectives needed.

Per-core pipeline (per batch, per n-tile of 128 rows):
  PE  : X tile stationary; one LDWEIGHTS feeds three matmuls:
          sl-raw   = X_tile^T @ (2 s C^T chunk)          [n,32]  (accum 4 d-chunks)
          gram     = X_tile^T @ X_tile                   [n,n]   (accum; diag = x2)
          XT chunk = X_tile^T @ I                        [n,128] (transpose)
  DVE : x2 = diag(gram) via tensor_tensor_reduce with identity mask;
        -sl = (-s)*x2 + sl_raw; then -sl-b with min-reduce -> -max
  ACT : expsl = exp(-( -sl-b ) + min) with accum -> denom; XT' = XT * (1/denom)
  PE  : E += expsl^T @ XT' ; colsumA += expsl^T @ (1/denom)
  DVE : E_final = E + colsumA*( -C )
"""

import os

import numpy as np

import bass_rust
import concourse.bass as bass
import concourse.mybir as mybir
import concourse.tile as tile
from concourse.masks import make_identity

# ---------------------------------------------------------------------------
# problem constants (hardcoded per contract)
B, D, H, W, K = 16, 512, 60, 60, 32
N = H * W  # 3600
N_CORES = 8
BPC = B // N_CORES  # batches per core = 2
DC = D // 128  # 4 d-chunks
NT = (N + 127) // 128  # 29 n-tiles (28 x 128 + 1 x 16)

FP = mybir.dt.float32
ALU = mybir.AluOpType
ACTF = mybir.ActivationFunctionType


def _patched_drain_and_barrier(self, tick_clock, wait_clock):
    # This walrus build accepts only ONE sync wait per instruction; the stock
    # TileContext exit emits a single drain carrying one wait per trailing
    # proc. Split it into a chain of single-wait drains.
    from concourse.vector_clock import ScopedClock

    drain_inst = self.nc.sync.drain()
    wait_clock.add_sem_waits(
        drain_inst.ins, ScopedClock({None: tick_clock.global_clock})
    )
    si = drain_inst.ins.sync_info
    waits = list(si.on_wait) if si is not None else []
    if len(waits) > 1:
        drain_inst.ins.sync_info = bass_rust.SyncInfo(
            on_wait=[waits[0]], on_update=list(si.on_update)
        )
        for w in waits[1:]:
            d2 = self.nc.sync.drain()
            d2.ins.sync_info = bass_rust.SyncInfo(on_wait=[w], on_update=[])
    self.nc.all_engine_barrier()
    assert self.sems is not None
    popped = self.nc._tile_sem_poison_stack.pop()
    assert popped is self._sem_poison
    self.nc.clear_and_free_semaphores(list(self.sems.allocated().values()))
    self.nc.all_engine_barrier()


tile.TileContext._drain_and_barrier = _patched_drain_and_barrier


def _split_multiwaits(obj):
    """Walk BIR JSON; any instruction with >1 on_wait gets the extra waits
    hoisted onto same-engine EventSemaphore carriers inserted before it."""
    counter = [0]

    def fix_list(insts):
        out = []
        for inst in insts:
            si = inst.get("sync_info") if isinstance(inst, dict) else None
            waits = (si or {}).get("on_wait") or []
            if len(waits) > 1:
                for w in waits[:-1]:
                    counter[0] += 1
                    out.append(
                        {
                            "debug": inst.get("debug", 0),
                            "engine": inst["engine"],
                            "ins": [],
                            "name": f"{inst['name']}-smw{counter[0]}",
                            "opcode": "EventSemaphore",
                            "outs": [],
                            "sync_info": {"on_update": [], "on_wait": [w]},
                        }
                    )
                si["on_wait"] = [waits[-1]]
            out.append(inst)
        return out

    def walk(o):
        if isinstance(o, dict):
            for k, v in o.items():
                if k == "instructions" and isinstance(v, list):
                    o[k] = fix_list(v)
                else:
                    walk(v)
        elif isinstance(o, list):
            for v in o:
                walk(v)

    walk(obj)
    return counter[0]


def _install_compile_patch():
    import json as _json

    from concourse import bass2jax, bass_utils

    if getattr(bass2jax, "_smw_patch", False):
        return
    _orig = bass_utils.compile_bir_kernel

    def _patched(bir_json, tmpdir, neff_name="file.neff"):
        d = _json.loads(bir_json)
        n = _split_multiwaits(d)
        if n:
            bir_json = _json.dumps(d).encode()
        return _orig(bir_json, tmpdir, neff_name=neff_name)

    bass2jax.compile_bir_kernel = _patched
    bass2jax._smw_patch = True


_install_compile_patch()


def build(reps: int = 1) -> bass.Bass:
    """Build the per-core Bass program. `reps` repeats the whole computation
    (including input DMA) for timing; outputs are identical every rep."""
    nc = bass.Bass()

    x_d = nc.dram_tensor("x", (BPC, D, N), FP, kind="ExternalInput")
    rsl_d = nc.dram_tensor("rsl", (D, K), FP, kind="ExternalInput")
    negs_d = nc.dram_tensor("negs", (K,), FP, kind="ExternalInput")
    bvec_d = nc.dram_tensor("bvec", (K,), FP, kind="ExternalInput")
    cneg_d = nc.dram_tensor("cneg", (K, D), FP, kind="ExternalInput")
    e_d = nc.dram_tensor("e", (BPC, K, D), FP, kind="ExternalOutput")

    with tile.TileContext(nc) as tc:
        with (
            tc.tile_pool(name="singles", bufs=1) as singles,
            tc.tile_pool(name="xpool", bufs=2) as xpool,
            tc.tile_pool(name="psum_sg", bufs=3, space="PSUM") as psum_sg,
            tc.tile_pool(name="psum_xt", bufs=3, space="PSUM") as psum_xt,
            tc.tile_pool(name="psum_acc", bufs=1, space="PSUM") as psum_acc,
            tc.tile_pool(name="small", bufs=8) as small,
            tc.tile_pool(name="scrp", bufs=2) as scrp,
            tc.tile_pool(name="slp", bufs=4) as slp,
            tc.tile_pool(name="ep", bufs=4) as ep,
            tc.tile_pool(name="xtp", bufs=4) as xtp,
            tc.tile_pool(name="outp", bufs=2) as outp,
        ):
            ident = singles.tile([128, 128], FP)
            make_identity(nc, ident)

            rsl_sb = singles.tile([128, DC, K], FP)
            nc.gpsimd.dma_start(
                out=rsl_sb, in_=rsl_d[:, :].rearrange("(c p) k -> p c k", p=128)
            )
            negs_sb = singles.tile([128, K], FP)
            nc.gpsimd.dma_start(
                out=negs_sb,
                in_=bass.AP(
                    tensor=negs_d[:].tensor,
                    offset=negs_d[:].offset,
                    ap=[[0, 128], [1, K]],
                ),
            )
            brow_sb = singles.tile([128, K], FP)
            nc.gpsimd.dma_start(
                out=brow_sb,
                in_=bass.AP(
                    tensor=bvec_d[:].tensor,
                    offset=bvec_d[:].offset,
                    ap=[[0, 128], [1, K]],
                ),
            )
            cneg_sb = singles.tile([K, D], FP)
            nc.gpsimd.dma_start(out=cneg_sb, in_=cneg_d[:, :])

            for _rep in range(reps):
                for b in range(BPC):
                    xn = xpool.tile([128, DC, N], FP, tag="xn")
                    # one DMA per d-chunk -> parallel queues, 1.8 MB each
                    for dc in range(DC):
                        nc.gpsimd.dma_start(
                            out=xn[:, dc, :],
                            in_=x_d[b, dc * 128 : (dc + 1) * 128, :],
                        )

                    psE = psum_acc.tile([K, D], FP, tag="psE")
                    psCS = psum_acc.tile([K, 1], FP, tag="psCS")

                    for t in range(NT):
                        off = t * 128
                        nt = min(128, N - off)
                        psg = psum_sg.tile([128, 160], FP, tag="psg")
                        pxt = psum_xt.tile([128, 512], FP, tag="pxt")
                        for dc in range(DC):
                            lhsT = xn[:, dc, off : off + nt]
                            nc.tensor.matmul(
                                psg[:nt, 0:K],
                                lhsT,
                                rsl_sb[:, dc, :],
                                start=(dc == 0),
                                stop=(dc == DC - 1),
                            )
                            nc.tensor.matmul(
                                psg[:nt, 32 : 32 + nt],
                                lhsT,
                                lhsT,
                                start=(dc == 0),
                                stop=(dc == DC - 1),
                            )
                            nc.tensor.matmul(
                                pxt[:nt, dc * 128 : dc * 128 + 128],
                                lhsT,
                                ident[:, :],
                                start=True,
                                stop=True,
                            )

                        # x2 = diag(gram) via mask-mult with row-sum accumulate
                        x2c = small.tile([128, 1], FP, tag="x2c")
                        scr = scrp.tile([128, 128], FP, tag="scr")
                        nc.vector.scalar_tensor_tensor(
                            out=scr[:nt, :nt],
                            in0=psg[:nt, 32 : 32 + nt],
                            scalar=0.0,
                            in1=ident[:nt, :nt],
                            op0=ALU.bypass,
                            op1=ALU.mult,
                            accum_out=x2c[:nt],
                        )
                        # -sl (without b term) = (-s)*x2 + sl_raw
                        sl1 = slp.tile([128, K], FP, tag="sl1")
                        nc.vector.scalar_tensor_tensor(
                            out=sl1[:nt],
                            in0=negs_sb[:nt],
                            scalar=x2c[:nt],
                            in1=psg[:nt, 0:K],
                            op0=ALU.mult,
                            op1=ALU.add,
                        )
                        # sl2 = -sl_full ; minv = min(sl2) = -max(sl_full)
                        sl2 = slp.tile([128, K], FP, tag="sl2")
                        minv = small.tile([128, 1], FP, tag="minv")
                        nc.vector.tensor_tensor(
                            out=sl2[:nt],
                            in0=sl1[:nt],
                            in1=brow_sb[:nt],
                            op=ALU.subtract,
                        )
                        nc.vector.tensor_reduce(
                            out=minv[:nt],
                            in_=sl2[:nt],
                            axis=mybir.AxisListType.X,
                            op=ALU.min,
                        )
                        # expsl = exp(sl_full - max) ; denom = sum_k
                        expsl = ep.tile([128, K], FP, tag="expsl")
                        den = small.tile([128, 1], FP, tag="den")
                        nc.scalar.activation(
                            out=expsl[:nt],
                            in_=sl2[:nt],
                            func=ACTF.Exp,
                            scale=-1.0,
                            bias=minv[:nt],
                            accum_out=den[:nt],
                        )
                        rcol = small.tile([128, 1], FP, tag="rcol")
                        nc.vector.reciprocal(rcol[:nt], den[:nt])
                        # XT' = XT * r (r applied to rows of XT instead of A)
                        xts = xtp.tile([128, 512], FP, tag="xts")
                        nc.scalar.activation(
                            out=xts[:nt],
                            in_=pxt[:nt],
                            func=ACTF.Copy,
                            scale=rcol[:nt],
                        )
                        # E += expsl^T @ XT' ; colsumA += expsl^T @ r
                        nc.tensor.matmul(
                            psE[:, :],
                            expsl[:nt, :],
                            xts[:nt, :],
                            start=(t == 0),
                            stop=(t == NT - 1),
                        )
                        nc.tensor.matmul(
                            psCS[:, :],
                            expsl[:nt, :],
                            rcol[:nt, :],
                            start=(t == 0),
                            stop=(t == NT - 1),
                        )

                    e_sb = outp.tile([K, D], FP, tag="e_sb")
                    nc.vector.scalar_tensor_tensor(
                        out=e_sb,
                        in0=cneg_sb,
                        scalar=psCS[:, :],
                        in1=psE[:, :],
                        op0=ALU.mult,
                        op1=ALU.add,
                    )
                    nc.gpsimd.dma_start(out=e_d[b, :, :], in_=e_sb)

    return nc


# ---------------------------------------------------------------------------
# host side


def _host_inputs(X, codewords, scale):
    Xr = np.ascontiguousarray(X.reshape(B, D, N))
    scale = scale.astype(np.float32)
    codewords = codewords.astype(np.float32)
    rsl = np.ascontiguousarray((2.0 * scale[:, None] * codewords).T)  # (D, K)
    negs = np.ascontiguousarray(-scale)
    bvec = np.ascontiguousarray(scale * (codewords * codewords).sum(axis=1))
    cneg = np.ascontiguousarray(-codewords)
    in_maps = []
    for c in range(N_CORES):
        in_maps.append(
            {
                "x": Xr[c * BPC : (c + 1) * BPC],
                "rsl": rsl,
                "negs": negs,
                "bvec": bvec,
                "cneg": cneg,
            }
        )
    return in_maps


class Runner:
    """jit-once / call-many executor for the SPMD kernel on 8 cores."""

    def __init__(self, reps: int = 1):
        import jax
        import numpy as np
        from jax.sharding import Mesh, NamedSharding, PartitionSpec
        from jax.experimental.shard_map import shard_map

        from concourse import bass2jax

        self.jax = jax
        nc = build(reps)
        bass2jax.install_neuronx_cc_hook()

        partition_name = (
            nc.partition_id_tensor.name if nc.partition_id_tensor else None
        )
        in_names, out_names, out_avals, zero_outs = [], [], [], []
        for alloc in nc.m.functions[0].allocations:
            if not isinstance(alloc, mybir.MemoryLocationSet):
                continue
            name = alloc.memorylocations[0].name
            if alloc.kind == "ExternalInput":
                if name != partition_name:
                    in_names.append(name)
            elif alloc.kind == "ExternalOutput":
                shape = tuple(alloc.tensor_shape)
                dt = mybir.dt.np(alloc.dtype)
                out_names.append(name)
                out_avals.append(
                    jax.core.ShapedArray(shape, dt)
                )
                zero_outs.append(np.zeros(shape, dt))
        self.in_names = list(in_names)
        self.out_names = out_names
        self.n_params = len(in_names)
        all_in_names = in_names + out_names
        if partition_name is not None:
            all_in_names.append(partition_name)

        def _body(*args):
            operands = list(args)
            if partition_name is not None:
                operands.append(bass2jax.partition_id_tensor())
            outs = bass2jax._bass_exec_p.bind(
                *operands,
                out_avals=tuple(out_avals),
                in_names=tuple(all_in_names),
                out_names=tuple(out_names),
                lowering_input_output_aliases=(),
                sim_require_finite=True,
                sim_require_nnan=True,
                nc=nc,
            )
            return tuple(outs)

        devices = jax.devices()[:N_CORES]
        self.mesh = Mesh(np.asarray(devices), ("core",))
        nin = self.n_params + len(out_names)
        self.fn = jax.jit(
            shard_map(
                _body,
                mesh=self.mesh,
                in_specs=(PartitionSpec("core"),) * nin,
                out_specs=(PartitionSpec("core"),) * len(out_names),
                check_rep=False,
            ),
            keep_unused=True,
        )
        self.sharding = NamedSharding(self.mesh, PartitionSpec("core"))
        self.zero_outs = zero_outs
        self._dev_args = None

    def put(self, in_maps):
        import jax

        concat = [
            np.concatenate([np.asarray(m[name]) for m in in_maps], axis=0)
            for name in self.in_names
        ]
        concat += [
            np.zeros((N_CORES * z.shape[0], *z.shape[1:]), z.dtype)
            for z in self.zero_outs
        ]
        self._dev_args = [jax.device_put(a, self.sharding) for a in concat]

    def run(self):
        outs = self.fn(*self._dev_args)
        self.jax.block_until_ready(outs)
        return outs

    def run_numpy(self):
        outs = self.run()
        res = []
        for c in range(N_CORES):
            res.append(
                {
                    name: np.asarray(outs[i]).reshape(
                        N_CORES, *self.zero_outs[i].shape
                    )[c]
                    for i, name in enumerate(self.out_names)
                }
            )
        return res


_RUNNER = None


def kernel(**inputs) -> np.ndarray:
    global _RUNNER
    X = np.asarray(inputs["X"], dtype=np.float32)
    codewords = np.asarray(inputs["codewords"], dtype=np.float32)
    scale = np.asarray(inputs["scale"], dtype=np.float32)
    if _RUNNER is None:
        _RUNNER = Runner(reps=1)
    _RUNNER.put(_host_inputs(X, codewords, scale))
    res = _RUNNER.run_numpy()
    E = np.concatenate([res[c]["e"] for c in range(N_CORES)], axis=0)
    return E.astype(np.float32)
